# revision 1
# baseline (speedup 1.0000x reference)
"""Causal attention (LN -> QKV -> 16-head causal attn -> out-proj) on 8 TRN2 cores.

Sharding: core c = (batch b=c//4, head-group g=c%4). Each core runs its batch's
LayerNorm + a 4-head slice of QKV / attention / out-projection. The out-proj
partials (column-split over the inner dim) are summed on the host per batch.

Device layout notes (per core):
  xnT  4x [128, 8, 512] bf16  normalized input, transposed (DIM on partitions),
                              split by seq-quarter so QKV pipelines into LN
  QT/KT pair tiles [128, 2048] bf16  (two heads stacked: head-dim on partitions)
  V    [128, 16, 4, 65] bf16  natural keys-on-partitions layout; 65th col = 1.0
                              so PV's lhsT also accumulates softmax denominators
  S^T  per (head-pair, q-half, key-block): psum [128 keys, 1024 q]; the two
       heads of a pair run concurrently via tile_position row-packing (d=64).
       exp on ScalarE straight from PSUM; causal staircase = skip fully masked
       regions + one DVE affine_select on the diagonal 128x128 block.
  outT psum [65, 1024]: rows 0..63 = unnormalized head output, row 64 = softmax
       denominators. Denominators land in a [4, 2048] SBUF tile; per-pair
       normalization (recip = exp(-ln d), DRAM-broadcast, DVE multiply) runs
       overlapped with the next pair's attention.
"""

import numpy as np

import concourse.bass as bass
import concourse.mybir as mybir
import concourse.tile as tile
from concourse import bacc
from concourse.bass_utils import run_bass_kernel_spmd
from concourse.masks import make_identity

B, N, DIM, HEADS, DIM_HEAD = 2, 2048, 1024, 16, 64
INNER = HEADS * DIM_HEAD
H_LOC = 4                      # heads per core
N_CORES = 8
P = 128
NB = N // P                    # 16 seq blocks
KB = DIM // P                  # 8 dim blocks
QT = 512                       # psum-bank-sized q tile
HALF = 1024                    # q span per S^T psum tile
SCALE = DIM_HEAD ** -0.5
LN_EPS = 1e-5

F32 = mybir.dt.float32
BF16 = mybir.dt.bfloat16
AF = mybir.ActivationFunctionType
ALU = mybir.AluOpType


def build_nc():
    from contextlib import ExitStack

    nc = bacc.Bacc(None, target_bir_lowering=False, debug=False)

    x_d = nc.dram_tensor("x", [N, DIM], F32, kind="ExternalInput")
    wq_d = nc.dram_tensor("wq", [DIM, H_LOC * DIM_HEAD], F32, kind="ExternalInput")
    wk_d = nc.dram_tensor("wk", [DIM, H_LOC * DIM_HEAD], F32, kind="ExternalInput")
    wv_d = nc.dram_tensor("wv", [DIM, H_LOC * DIM_HEAD], F32, kind="ExternalInput")
    wo_d = nc.dram_tensor("wo", [H_LOC * DIM_HEAD, DIM], F32, kind="ExternalInput")
    bq_d = nc.dram_tensor("bq", [P, 2], F32, kind="ExternalInput")
    bk_d = nc.dram_tensor("bk", [P, 2], F32, kind="ExternalInput")
    bv_d = nc.dram_tensor("bv", [1, H_LOC * DIM_HEAD], F32, kind="ExternalInput")
    out_d = nc.dram_tensor("out", [N, DIM], F32, kind="ExternalOutput")

    with tile.TileContext(nc) as tc:
        ctx = ExitStack()
        with ctx:
            const = ctx.enter_context(tc.tile_pool(name="const", bufs=1))
            persist = ctx.enter_context(tc.tile_pool(name="persist", bufs=1))
            wstage = ctx.enter_context(tc.tile_pool(name="wstage", bufs=2))
            xpool = ctx.enter_context(tc.tile_pool(name="xpool", bufs=5))
            xnpool = ctx.enter_context(tc.tile_pool(name="xnpool", bufs=4))
            stat = ctx.enter_context(tc.tile_pool(name="stat", bufs=8))
            expp = ctx.enter_context(tc.tile_pool(name="expp", bufs=3))
            smalls = ctx.enter_context(tc.tile_pool(name="smalls", bufs=2))
            rbcp = ctx.enter_context(tc.tile_pool(name="rbcp", bufs=2))
            dramp = ctx.enter_context(tc.tile_pool(name="dramp", bufs=2, space="DRAM"))
            stage = ctx.enter_context(tc.tile_pool(name="stage", bufs=3))

            # ---- constants ----
            ident = const.tile([P, P], BF16, tag="ident")
            make_identity(nc, ident)
            # keep-mask for the causal diagonal block: tri[k, q] = (k <= q)
            tri = const.tile([P, P], BF16, tag="tri")
            nc.gpsimd.memset(tri[:], 0.0)
            nc.gpsimd.affine_select(
                out=tri[:], in_=tri[:], compare_op=ALU.is_gt, fill=1.0,
                base=0, channel_multiplier=1, pattern=[[-1, P]],
            )
            eps_t = const.tile([P, 1], F32, tag="eps")
            nc.vector.memset(eps_t, LN_EPS)
            bq_sb = const.tile([P, 2], F32, tag="bq")
            nc.sync.dma_start(bq_sb[:], bq_d[:])
            bk_sb = const.tile([P, 2], F32, tag="bk")
            nc.sync.dma_start(bk_sb[:], bk_d[:])
            bv_sb = const.tile([P, H_LOC, DIM_HEAD], F32, tag="bv")
            nc.sync.dma_start(
                bv_sb[:],
                bv_d[:].rearrange("o (h d) -> o h d", h=H_LOC)
                .to_broadcast((P, H_LOC, DIM_HEAD)),
            )

            xnT = [persist.tile([P, KB, 4 * P], BF16, tag=f"xnT{q}", name=f"xnT{q}")
                   for q in range(4)]
            QTt = [persist.tile([P, N], BF16, tag=f"qt{p_}", name=f"qt{p_}")
                   for p_ in range(2)]
            KTt = [persist.tile([P, N], BF16, tag=f"kt{p_}", name=f"kt{p_}")
                   for p_ in range(2)]
            Vt = persist.tile([P, NB, H_LOC, DIM_HEAD + 1], BF16, tag="v")
            nc.gpsimd.memset(Vt[:], 1.0)  # 65th column stays 1.0 -> denominators
            outT = [[persist.tile([P, HALF], BF16, tag=f"outT{p_}_{q_}",
                                  name=f"outT{p_}_{q_}") for q_ in range(2)]
                    for p_ in range(2)]

            # ---- phase A: LN -> transpose -> QKV -> V (interleaved) ----
            psA_cm = tc.tile_pool(name="psA", bufs=4, space="PSUM")
            psA = psA_cm.__enter__()

            def load_w_bf(dram, shape3, tag, eng):
                st = wstage.tile(shape3, F32, tag="wst", name=f"wst_{tag}")
                nc.sync.dma_start(st[:], dram[:].rearrange("(kb p) m -> p kb m", p=P))
                bf = persist.tile(shape3, BF16, tag=tag, name=f"bf_{tag}")
                if eng == "act":
                    nc.scalar.copy(bf[:], st[:])
                elif eng == "dve":
                    nc.vector.tensor_copy(bf[:], st[:])
                else:
                    nc.gpsimd.tensor_copy(bf[:], st[:])
                return bf

            wq_bf = load_w_bf(wq_d, [P, KB, H_LOC * DIM_HEAD], "wq", "act")
            wk_bf = load_w_bf(wk_d, [P, KB, H_LOC * DIM_HEAD], "wk", "act")
            wv_bf = load_w_bf(wv_d, [P, KB, H_LOC * DIM_HEAD], "wv", "dve")
            wo_bf = load_w_bf(wo_d, [P, 2, DIM], "wo", "gps")

            def emit_qkv_st(st):
                for (wt, bias_sb, dstt) in ((wq_bf, bq_sb, QTt), (wk_bf, bk_sb, KTt)):
                    for pr in range(2):
                        ps = psA.tile([P, 512], F32, tag="ps")
                        for kb in range(KB):
                            nc.tensor.matmul(
                                ps[:],
                                wt[:, kb, pr * P:(pr + 1) * P],
                                xnT[st][:, kb, :],
                                start=(kb == 0), stop=(kb == KB - 1),
                            )
                        nc.vector.tensor_scalar_add(
                            dstt[pr][:, st * 512:(st + 1) * 512], ps[:],
                            bias_sb[:, pr:pr + 1],
                        )

            for sb in range(NB):
                x_t = xpool.tile([P, DIM], F32, tag="x")
                nc.sync.dma_start(x_t[:], x_d[sb * P:(sb + 1) * P, :])

                stats = stat.tile([P, 2, 6], F32, tag="bnst")
                x3 = x_t[:].rearrange("p (a f) -> p a f", a=2)
                for a in range(2):
                    nc.vector.bn_stats(stats[:, a, :], x3[:, a, :])
                mv = stat.tile([P, 2], F32, tag="mv")
                nc.vector.bn_aggr(mv[:], stats[:])
                rstd = stat.tile([P, 1], F32, tag="rstd")
                nc.scalar.activation(rstd[:], mv[:, 1:2], AF.Sqrt, bias=eps_t[:])
                nc.vector.reciprocal(rstd[:], rstd[:])
                # nmrs = -mean * rstd  -> xn = x*rstd + nmrs on ScalarE
                nmrs = stat.tile([P, 1], F32, tag="nmrs")
                nc.vector.tensor_scalar(
                    nmrs[:], mv[:, 0:1], rstd[:], -1.0, ALU.mult, ALU.mult
                )
                xn_bf = xnpool.tile([P, DIM], BF16, tag="xn")
                nc.scalar.activation(
                    xn_bf[:], x_t[:], AF.Identity, bias=nmrs[:], scale=rstd[:]
                )

                # transpose this seq block: 8 dim-blocks via PE, 2 psum tiles
                for half in range(2):
                    ps = psA.tile([P, 512], F32, tag="ps")
                    for j in range(4):
                        kb = half * 4 + j
                        nc.tensor.matmul(
                            ps[:, j * P:(j + 1) * P],
                            xn_bf[:, kb * P:(kb + 1) * P],
                            ident[:],
                            start=True, stop=True,
                        )
                    dst = xnT[sb // 4][:, half * 4:(half + 1) * 4,
                                       (sb % 4) * P:(sb % 4 + 1) * P]
                    src = ps[:].rearrange("p (a f) -> p a f", a=4)
                    if half == 0:
                        nc.scalar.copy(dst, src)
                    else:
                        nc.vector.tensor_copy(dst, src)

                # V for this seq block
                ps = psA.tile([P, 512], F32, tag="ps")
                psv = ps[:, :H_LOC * DIM_HEAD]
                for kb in range(KB):
                    nc.tensor.matmul(
                        psv,
                        xnT[sb // 4][:, kb, (sb % 4) * P:(sb % 4 + 1) * P],
                        wv_bf[:, kb, :],
                        start=(kb == 0), stop=(kb == KB - 1),
                    )
                nc.vector.tensor_tensor(
                    Vt[:, sb, :, :DIM_HEAD],
                    psv.rearrange("p (h d) -> p h d", h=H_LOC),
                    bv_sb[:],
                    ALU.add,
                )

                if sb % 4 == 3:
                    emit_qkv_st(sb // 4)

            psA_cm.__exit__(None, None, None)

            # ---- phase B: attention, head pairs via tile_position packing ----
            ctx2 = ExitStack()
            with ctx2:
                psS = ctx2.enter_context(tc.tile_pool(name="psS", bufs=1, space="PSUM"))
                psO = ctx2.enter_context(tc.tile_pool(name="psO", bufs=1, space="PSUM"))

                for pr in range(2):
                    for qh in range(2):
                        qs, qe = qh * HALF, (qh + 1) * HALF
                        ps_o = [psO.tile([DIM_HEAD + 1, HALF], F32,
                                         tag=f"po{hh}", name=f"po{hh}_{pr}_{qh}")
                                for hh in range(2)]
                        for kb in range(NB):
                            qlo = kb * P
                            if qlo >= qe:
                                break
                            s_ps = [psS.tile([P, HALF], F32, tag=f"ps_s{hh}",
                                             name=f"ps_s{hh}_{pr}_{qh}_{kb}")
                                    for hh in range(2)]
                            for qt in range(qs // QT, qe // QT):
                                rs, re = qt * QT, (qt + 1) * QT
                                if re <= qlo:
                                    continue
                                for hh in range(2):
                                    po = hh * DIM_HEAD
                                    nc.tensor.matmul(
                                        s_ps[hh][:, rs - qs:re - qs],
                                        KTt[pr][po:po + DIM_HEAD, qlo:qlo + P],
                                        QTt[pr][po:po + DIM_HEAD, rs:re],
                                        start=True, stop=True,
                                        tile_position=(po, 0),
                                    )
                            vstart = max(qlo, qs)
                            exs = []
                            for hh in range(2):
                                ex = expp.tile([P, HALF], BF16, tag=f"ex{hh}",
                                               name=f"ex{hh}_{pr}_{qh}_{kb}")
                                nc.scalar.activation(
                                    ex[:, vstart - qs:], s_ps[hh][:, vstart - qs:],
                                    AF.Exp,
                                )
                                if qlo >= qs:
                                    nc.vector.tensor_tensor(
                                        ex[:, qlo - qs:qlo - qs + P],
                                        ex[:, qlo - qs:qlo - qs + P],
                                        tri[:],
                                        ALU.mult,
                                    )
                                exs.append(ex)
                            for qt in range(qs // QT, qe // QT):
                                rs, re = qt * QT, (qt + 1) * QT
                                if re <= qlo:
                                    continue
                                cs = max(qlo, rs)
                                for hh in range(2):
                                    nc.tensor.matmul(
                                        ps_o[hh][:, cs - qs:re - qs],
                                        Vt[:, kb, 2 * pr + hh, :],
                                        exs[hh][:, cs - qs:re - qs],
                                        start=(kb == 0),
                                        stop=(kb == min(NB - 1, re // P - 1)),
                                    )
                        # evacuate unnormalized output (DVE) + denom row (DMA)
                        # and normalize this (pair, q-half) immediately:
                        # reciprocal on DVE in [128,8] layout via DRAM shuffles.
                        recip_bc = rbcp.tile([P, HALF], F32, tag="rbc",
                                             name=f"rbc{pr}_{qh}")
                        for hh in range(2):
                            nc.vector.tensor_copy(
                                outT[pr][qh][hh * DIM_HEAD:(hh + 1) * DIM_HEAD, :],
                                ps_o[hh][:DIM_HEAD, :],
                            )
                            dr = stat.tile([1, HALF], F32, tag="denrow",
                                           name=f"dr{pr}_{qh}_{hh}")
                            nc.vector.tensor_copy(
                                dr[:], ps_o[hh][DIM_HEAD:DIM_HEAD + 1, :]
                            )
                            da = dramp.tile([1, HALF], F32, tag="da",
                                            name=f"da{pr}_{qh}_{hh}")
                            nc.sync.dma_start(da[:], dr[:])
                            denc = stat.tile([P, HALF // P], F32, tag="denc",
                                             name=f"denc{pr}_{qh}_{hh}")
                            nc.sync.dma_start(
                                denc[:],
                                da[0, :].rearrange("(p o) -> p o", o=HALF // P),
                            )
                            nc.vector.reciprocal(denc[:], denc[:])
                            db = dramp.tile([1, HALF], F32, tag="db",
                                            name=f"db{pr}_{qh}_{hh}")
                            nc.sync.dma_start(
                                db[0, :].rearrange("(p o) -> p o", o=HALF // P),
                                denc[:],
                            )
                            nc.sync.dma_start(
                                recip_bc[hh * DIM_HEAD:(hh + 1) * DIM_HEAD, :],
                                db[:].to_broadcast((DIM_HEAD, HALF)),
                            )
                        nc.vector.tensor_tensor(
                            outT[pr][qh][:], outT[pr][qh][:], recip_bc[:], ALU.mult
                        )

            # ---- phase C: out projection ----
            psP = ctx.enter_context(tc.tile_pool(name="psP", bufs=3, space="PSUM"))
            for qb in range(NB):
                ps = psP.tile([P, 2, 512], F32, tag="pp")
                for nt in range(2):
                    for pb in range(2):
                        nc.tensor.matmul(
                            ps[:, nt, :],
                            outT[pb][qb // 8][:, (qb % 8) * P:(qb % 8 + 1) * P],
                            wo_bf[:, pb, nt * 512:(nt + 1) * 512],
                            start=(pb == 0), stop=(pb == 1),
                        )
                so = stage.tile([P, DIM], F32, tag="so")
                if qb % 2 == 0:
                    nc.scalar.copy(so[:], ps[:].rearrange("p a f -> p (a f)"))
                else:
                    nc.vector.tensor_copy(so[:], ps[:].rearrange("p a f -> p (a f)"))
                nc.sync.dma_start(out_d[qb * P:(qb + 1) * P, :], so[:])

    nc.compile()
    return nc


def make_in_maps(x, ln_w, ln_b, w_qkv, w_out):
    x = np.asarray(x, np.float32)
    ln_w = np.asarray(ln_w, np.float32)
    ln_b = np.asarray(ln_b, np.float32)
    w_qkv = np.asarray(w_qkv, np.float32)
    w_out = np.asarray(w_out, np.float32)

    in_maps = []
    for c in range(N_CORES):
        b, g = c // 4, c % 4
        cols = np.arange(4 * g * DIM_HEAD, (4 * g + H_LOC) * DIM_HEAD)
        wq_s = w_qkv[:, cols]
        wk_s = w_qkv[:, INNER + cols]
        wv_s = w_qkv[:, 2 * INNER + cols]
        wq = np.ascontiguousarray(ln_w[:, None] * wq_s * SCALE)
        wk = np.ascontiguousarray(ln_w[:, None] * wk_s)
        wv = np.ascontiguousarray(ln_w[:, None] * wv_s)
        bq = (ln_b @ wq_s) * SCALE
        bk = ln_b @ wk_s
        bv = ln_b @ wv_s
        in_maps.append({
            "x": np.ascontiguousarray(x[b]),
            "wq": wq, "wk": wk, "wv": wv,
            "wo": np.ascontiguousarray(w_out[cols, :]),
            "bq": np.ascontiguousarray(bq.reshape(2, P).T),
            "bk": np.ascontiguousarray(bk.reshape(2, P).T),
            "bv": bv.reshape(1, H_LOC * DIM_HEAD),
        })
    return in_maps


_NC_CACHE = []


def kernel(x, ln_w, ln_b, w_qkv, w_out):
    in_maps = make_in_maps(x, ln_w, ln_b, w_qkv, w_out)
    if not _NC_CACHE:
        _NC_CACHE.append(build_nc())
    nc = _NC_CACHE[0]
    res = run_bass_kernel_spmd(nc, in_maps, list(range(N_CORES))).results
    out = np.zeros((B, N, DIM), np.float32)
    for c in range(N_CORES):
        out[c // 4] += res[c]["out"]
    return out



# revision 6
# speedup vs baseline: 1.1914x; 1.1914x over previous
"""Causal attention (LN -> QKV -> 16-head causal attn -> out-proj) on 8 TRN2 cores.

Sharding: core c = (batch b=c//4, head-group g=c%4); each core does its batch's
LayerNorm + 4 heads of QKV/attention/out-proj; host sums the 4 column-split
out-proj partials per batch.

Single fused pipeline per core (vs. the old 3-phase structure):
  - x arrives bf16 (halves input DMA); LN fully on DVE (bn_stats + Quake rsqrt
    via int32 bit tricks) so ScalarE runs ONLY exp (ACT table loaded once).
  - xn transposed by the DMA XBAR (SBUF->SBUF, d = 128*kb+p chunk order), not
    the PE array; weights arrive pre-chunked/pre-folded (ln_w, scale) in bf16.
  - attention in (pr, qt) groups of 512 queries x 2 heads: S^T for both heads
    lands in ONE [128,1024] psum tile (two 64-contraction matmuls, operands in
    per-head [64,*] Q/K tiles at partitions 0-63) -> ONE exp ACTIVATE per
    (pr,qt,kb) over a [128,2,span] view; causal staircase skips masked blocks,
    one broadcast tri-multiply on the diagonal block.
  - V tiles carry 64 ones-columns so each PV matmul also lands 64 copies of the
    softmax denominator in psum rows 64:127: normalization = gpsimd copy +
    reciprocal_approx_fast + one DVE multiply, no DRAM round trips.
  - QKV for later seq quarters and the out-projection are emitted as "fillers"
    inside the attention kb loops so the PE never idles while ScalarE runs exp.
  PSUM: S [128,1024]x2bufs (4 banks) + psO 2x[128,512] (2) + general [128,512]
  x2bufs (2) = 8 banks exactly.
"""

import numpy as np
import ml_dtypes

import concourse.bass as bass
import concourse.mybir as mybir
import concourse.tile as tile
from concourse import bacc
from concourse.bass_utils import run_bass_kernel_spmd

B, N, DIM, HEADS, DIM_HEAD = 2, 2048, 1024, 16, 64
INNER = HEADS * DIM_HEAD
H_LOC = 4                      # heads per core
N_CORES = 8
P = 128
NB = N // P                    # 16 seq blocks
KB = DIM // P                  # 8 dim blocks
QT = 512                       # queries per attention group
SCALE = DIM_HEAD ** -0.5
LN_EPS = 1e-5
MAGIC = 0x5F3759DF

F32 = mybir.dt.float32
BF16 = mybir.dt.bfloat16
I32 = mybir.dt.int32
AF = mybir.ActivationFunctionType
ALU = mybir.AluOpType


def build_nc():
    from contextlib import ExitStack

    nc = bacc.Bacc(None, target_bir_lowering=False, debug=False)

    x_d = nc.dram_tensor("x", [N, DIM], BF16, kind="ExternalInput")
    wq_d = nc.dram_tensor("wq", [P, KB, 2 * P], BF16, kind="ExternalInput")
    wk_d = nc.dram_tensor("wk", [P, KB, 2 * P], BF16, kind="ExternalInput")
    wv_d = nc.dram_tensor("wv", [P, KB, 2 * P], BF16, kind="ExternalInput")
    wo_d = nc.dram_tensor("wo", [P, 2, DIM], BF16, kind="ExternalInput")
    bq_d = nc.dram_tensor("bq", [P, 2], F32, kind="ExternalInput")
    bk_d = nc.dram_tensor("bk", [P, 2], F32, kind="ExternalInput")
    bv_d = nc.dram_tensor("bv", [1, H_LOC, DIM_HEAD], F32, kind="ExternalInput")
    tri_d = nc.dram_tensor("tri", [P, P], BF16, kind="ExternalInput")
    out_d = nc.dram_tensor("out", [N, DIM], F32, kind="ExternalOutput")

    with tile.TileContext(nc) as tc:
        ctx = ExitStack()
        with ctx:
            const = ctx.enter_context(tc.tile_pool(name="const", bufs=1))
            persist = ctx.enter_context(tc.tile_pool(name="persist", bufs=1))
            xnpool = ctx.enter_context(tc.tile_pool(name="xnpool", bufs=2))
            statp = ctx.enter_context(tc.tile_pool(name="statp", bufs=2))
            expp = ctx.enter_context(tc.tile_pool(name="expp", bufs=3))
            dsb = ctx.enter_context(tc.tile_pool(name="dsb", bufs=2))
            stage = ctx.enter_context(tc.tile_pool(name="stage", bufs=3))
            psS = ctx.enter_context(tc.tile_pool(name="psS", bufs=2, space="PSUM"))
            psO = ctx.enter_context(tc.tile_pool(name="psO", bufs=1, space="PSUM"))
            psG = ctx.enter_context(tc.tile_pool(name="psG", bufs=2, space="PSUM"))

            # ---- prologue: exp table preload, weight/const/x DMAs ----
            dmy_f = const.tile([P, 1], F32, tag="dmy_f", name="dmy_f")
            nc.vector.memset(dmy_f, 0.0)
            dmy_b = const.tile([P, 1], BF16, tag="dmy_b", name="dmy_b")
            nc.scalar.activation(dmy_b[:], dmy_f[:], AF.Exp)

            wq_sb = persist.tile([P, KB, 2 * P], BF16, tag="wq", name="wq_sb")
            nc.sync.dma_start(wq_sb[:], wq_d[:])
            wk_sb = persist.tile([P, KB, 2 * P], BF16, tag="wk", name="wk_sb")
            nc.sync.dma_start(wk_sb[:], wk_d[:])
            wv_sb = persist.tile([P, KB, 2 * P], BF16, tag="wv", name="wv_sb")
            nc.sync.dma_start(wv_sb[:], wv_d[:])

            x_t = [persist.tile([P, DIM], BF16, tag=f"x{sb}", name=f"x{sb}")
                   for sb in range(NB)]
            for sb in range(NB):
                nc.sync.dma_start(x_t[sb][:], x_d[sb * P:(sb + 1) * P, :])

            wo_sb = persist.tile([P, 2, DIM], BF16, tag="wo", name="wo_sb")
            nc.sync.dma_start(wo_sb[:], wo_d[:])
            bq_sb = const.tile([P, 2], F32, tag="bq", name="bq_sb")
            nc.sync.dma_start(bq_sb[:], bq_d[:])
            bk_sb = const.tile([P, 2], F32, tag="bk", name="bk_sb")
            nc.sync.dma_start(bk_sb[:], bk_d[:])
            bv_sb = const.tile([P, H_LOC, DIM_HEAD], F32, tag="bv", name="bv_sb")
            nc.sync.dma_start(bv_sb[:], bv_d[:].to_broadcast((P, H_LOC, DIM_HEAD)))
            tri_t = const.tile([P, P], BF16, tag="tri", name="tri_t")
            nc.sync.dma_start(tri_t[:], tri_d[:])
            magic_t = const.tile([P, 4], I32, tag="magic", name="magic_t")
            nc.vector.memset(magic_t, MAGIC)

            # persistent activations
            xnT = [persist.tile([P, KB, 4 * P], BF16, tag=f"xnT{st}",
                                name=f"xnT{st}") for st in range(4)]
            QTh = [[persist.tile([DIM_HEAD, N], BF16, tag=f"qt{pr}{hh}",
                                 name=f"qt{pr}{hh}") for hh in range(2)]
                   for pr in range(2)]
            KTh = [[persist.tile([DIM_HEAD, N], BF16, tag=f"kt{pr}{hh}",
                                 name=f"kt{pr}{hh}") for hh in range(2)]
                   for pr in range(2)]
            Vt = persist.tile([P, NB, H_LOC, P], BF16, tag="v", name="Vt")
            nc.gpsimd.memset(Vt[:], 1.0)   # cols 64:128 stay 1.0 -> denominators
            outT = [persist.tile([P, N], BF16, tag=f"outT{pr}", name=f"outT{pr}")
                    for pr in range(2)]

            # ---- phase-A building blocks ----
            def emit_ln_st(st):
                """LN chain for seq quarter st: stats, batched rsqrt, xn, T-dma."""
                stp = statp.tile([P, 4, 2], F32, tag="stp", name=f"stp{st}")
                for j in range(4):
                    sb = st * 4 + j
                    st6 = statp.tile([P, 2, 6], F32, tag="st6", name=f"st6_{sb}")
                    x3 = x_t[sb][:].rearrange("p (a f) -> p a f", a=2)
                    for a in range(2):
                        nc.vector.bn_stats(st6[:, a, :], x3[:, a, :])
                    nc.vector.bn_aggr(stp[:, j, :], st6[:])
                # rstd = 1/sqrt(var+eps) via Quake + 2 Newton (all DVE)
                veps = statp.tile([P, 4], F32, tag="veps", name=f"veps{st}")
                nc.vector.tensor_scalar_add(veps[:], stp[:, :, 1], LN_EPS)
                iv = statp.tile([P, 4], I32, tag="iv", name=f"iv{st}")
                nc.vector.tensor_scalar(iv[:], veps[:].bitcast(I32), 1, None,
                                        ALU.logical_shift_right)
                y0i = statp.tile([P, 4], I32, tag="y0i", name=f"y0i{st}")
                nc.vector.tensor_tensor(y0i[:], magic_t[:], iv[:], ALU.subtract)
                t1 = statp.tile([P, 4], F32, tag="t1", name=f"t1{st}")
                rstd = statp.tile([P, 4], F32, tag="rstd", name=f"rstd{st}")
                for it in range(2):
                    src = y0i[:].bitcast(F32) if it == 0 else rstd[:]
                    nc.vector.tensor_tensor(t1[:], src, src, ALU.mult)
                    nc.vector.tensor_tensor(t1[:], t1[:], veps[:], ALU.mult)
                    nc.vector.tensor_scalar(t1[:], t1[:], -0.5, 1.5, ALU.mult,
                                            ALU.add)
                    nc.vector.tensor_tensor(rstd[:], src, t1[:], ALU.mult)
                nmrs = statp.tile([P, 4], F32, tag="nmrs", name=f"nmrs{st}")
                nc.vector.tensor_tensor(nmrs[:], stp[:, :, 0], rstd[:], ALU.mult)
                nc.vector.tensor_scalar_mul(nmrs[:], nmrs[:], -1.0)
                for j in range(4):
                    sb = st * 4 + j
                    xn = xnpool.tile([P, DIM], BF16, tag="xn", name=f"xn{sb}")
                    nc.vector.tensor_scalar(
                        xn[:], x_t[sb][:], rstd[:, j:j + 1], nmrs[:, j:j + 1],
                        ALU.mult, ALU.add,
                    )
                    nc.sync.dma_start(
                        xnT[st][:, :, j * P:(j + 1) * P], xn[:], transpose=True
                    )

            def emit_v(st, j):
                """V projection for seq block sb = 4*st+j (PE + gpsimd evac)."""
                sb = st * 4 + j
                gp = psG.tile([P, 512], F32, tag="gp", name=f"gpv{sb}")
                for kb in range(KB):
                    nc.tensor.matmul(
                        gp[:, 0:2 * P],
                        xnT[st][:, kb, j * P:(j + 1) * P],
                        wv_sb[:, kb, :],
                        start=(kb == 0), stop=(kb == KB - 1),
                    )
                nc.vector.tensor_tensor(
                    Vt[:, sb, :, 0:DIM_HEAD],
                    gp[:, 0:2 * P].rearrange("p (h d) -> p h d", h=H_LOC),
                    bv_sb[:],
                    ALU.add,
                )

            def emit_qk(st, wt, pr):
                """Q^T or K^T for quarter st, head pair pr (PE + DVE evac)."""
                w_sb, bias, dst = (
                    (wq_sb, bq_sb, QTh) if wt == 0 else (wk_sb, bk_sb, KTh)
                )
                gp = psG.tile([P, 512], F32, tag="gp", name=f"gpqk{st}{wt}{pr}")
                for kb in range(KB):
                    nc.tensor.matmul(
                        gp[:],
                        w_sb[:, kb, pr * P:(pr + 1) * P],
                        xnT[st][:, kb, :],
                        start=(kb == 0), stop=(kb == KB - 1),
                    )
                for hh in range(2):
                    nc.vector.tensor_scalar_add(
                        dst[pr][hh][:, st * 512:(st + 1) * 512],
                        gp[hh * DIM_HEAD:(hh + 1) * DIM_HEAD, :],
                        bias[hh * DIM_HEAD:(hh + 1) * DIM_HEAD, pr:pr + 1],
                    )

            def emit_outproj(qb):
                """Out-projection + store for query block qb (128 queries)."""
                so = stage.tile([P, DIM], F32, tag="so", name=f"so{qb}")
                for nt in range(2):
                    gp = psG.tile([P, 512], F32, tag="gp", name=f"gpo{qb}{nt}")
                    for pr in range(2):
                        nc.tensor.matmul(
                            gp[:],
                            outT[pr][:, qb * P:(qb + 1) * P],
                            wo_sb[:, pr, nt * 512:(nt + 1) * 512],
                            start=(pr == 0), stop=(pr == 1),
                        )
                    nc.vector.tensor_copy(so[:, nt * 512:(nt + 1) * 512], gp[:])
                nc.sync.dma_start(out_d[qb * P:(qb + 1) * P, :], so[:])

            # filler queue: (need_before_group_index, closure)
            fillers = []

            def pop_filler(gi, force=False):
                while fillers:
                    fillers.pop(0)[1]()
                    if not force:
                        break

            def drain_required(gi):
                while fillers and fillers[0][0] <= gi:
                    fillers.pop(0)[1]()

            # ---- attention group ----
            def attn_group(pr, qt, gi):
                drain_required(gi)
                qs = qt * QT
                last_kb = 4 * qt + 3
                po = [psO.tile([P, QT], F32, tag=f"po{hh}",
                               name=f"po{hh}_{pr}_{qt}") for hh in range(2)]
                sps_prev = None
                ex_prev = None
                voff_prev = 0

                def s_mm(kb):
                    voff = max(0, (kb - 4 * qt) * P)
                    sps = psS.tile([P, 2 * QT], F32, tag="sps",
                                   name=f"sps{pr}_{qt}_{kb}")
                    for hh in range(2):
                        nc.tensor.matmul(
                            sps[:, hh * QT + voff:(hh + 1) * QT],
                            KTh[pr][hh][:, kb * P:(kb + 1) * P],
                            QTh[pr][hh][:, qs + voff:qs + QT],
                            start=True, stop=True,
                        )
                    return sps, voff

                sps_prev, voff_prev = s_mm(0)
                for kb in range(last_kb + 1):
                    sps, voff = sps_prev, voff_prev
                    # exp for kb (ScalarE), next S (PE), then PV for kb (PE)
                    ex = expp.tile([P, 2 * QT], BF16, tag="ex",
                                   name=f"ex{pr}_{qt}_{kb}")
                    ex3 = ex[:].rearrange("p (h q) -> p h q", h=2)
                    sp3 = sps[:].rearrange("p (h q) -> p h q", h=2)
                    nc.scalar.activation(
                        ex3[:, :, voff:QT], sp3[:, :, voff:QT], AF.Exp
                    )
                    if kb < last_kb:
                        sps_prev, voff_prev = s_mm(kb + 1)
                    if kb >= 4 * qt:   # diagonal block: causal mask
                        tri_b = tri_t[:].rearrange(
                            "p (o q) -> p o q", o=1).to_broadcast((P, 2, P))
                        nc.vector.tensor_tensor(
                            ex3[:, :, voff:voff + P],
                            ex3[:, :, voff:voff + P],
                            tri_b,
                            ALU.mult,
                        )
                    for hh in range(2):
                        nc.tensor.matmul(
                            po[hh][:, voff:QT],
                            Vt[:, kb, 2 * pr + hh, :],
                            ex3[:, hh, voff:QT],
                            start=(kb == 0), stop=(kb == last_kb),
                        )
                    pop_filler(gi)
                # normalize: rows 64:127 of po hold the denominator
                for hh in range(2):
                    den = dsb.tile([DIM_HEAD, QT], F32, tag="den",
                                   name=f"den{pr}_{qt}_{hh}")
                    nc.vector.tensor_copy(den[:], po[hh][DIM_HEAD:P, :])
                    rcp = dsb.tile([DIM_HEAD, QT], F32, tag="rcp",
                                   name=f"rcp{pr}_{qt}_{hh}")
                    nc.vector.reciprocal_approx_fast(rcp[:], den[:])
                    nc.vector.tensor_tensor(
                        outT[pr][hh * DIM_HEAD:(hh + 1) * DIM_HEAD, qs:qs + QT],
                        po[hh][0:DIM_HEAD, :],
                        rcp[:],
                        ALU.mult,
                    )

            # ---- emission schedule ----
            emit_ln_st(0)
            for j in range(4):
                emit_v(0, j)
            for wt in range(2):
                for pr in range(2):
                    emit_qk(0, wt, pr)
            emit_ln_st(1)
            for j in range(4):
                emit_v(1, j)
            for wt in range(2):
                for pr in range(2):
                    emit_qk(1, wt, pr)

            attn_group(0, 0, 0)
            emit_ln_st(2)
            for j in range(4):
                fillers.append((4, (lambda st, j: lambda: emit_v(st, j))(2, j)))
            for wt in range(2):
                for pr in range(2):
                    fillers.append(
                        (4, (lambda a, b, c: lambda: emit_qk(a, b, c))(2, wt, pr)))
            attn_group(1, 0, 1)
            emit_ln_st(3)
            for j in range(4):
                fillers.append((6, (lambda st, j: lambda: emit_v(st, j))(3, j)))
            for wt in range(2):
                for pr in range(2):
                    fillers.append(
                        (6, (lambda a, b, c: lambda: emit_qk(a, b, c))(3, wt, pr)))
            for qb in range(4):
                fillers.append((99, (lambda q: lambda: emit_outproj(q))(qb)))
            attn_group(0, 1, 2)
            attn_group(1, 1, 3)
            for qb in range(4, 8):
                fillers.append((99, (lambda q: lambda: emit_outproj(q))(qb)))
            attn_group(0, 2, 4)
            attn_group(1, 2, 5)
            for qb in range(8, 12):
                fillers.append((99, (lambda q: lambda: emit_outproj(q))(qb)))
            attn_group(0, 3, 6)
            attn_group(1, 3, 7)
            for qb in range(12, 16):
                fillers.append((99, (lambda q: lambda: emit_outproj(q))(qb)))
            pop_filler(99, force=True)

    nc.compile()
    return nc


def make_in_maps(x, ln_w, ln_b, w_qkv, w_out):
    x = np.asarray(x, np.float32)
    ln_w = np.asarray(ln_w, np.float32)
    ln_b = np.asarray(ln_b, np.float32)
    w_qkv = np.asarray(w_qkv, np.float32)
    w_out = np.asarray(w_out, np.float32)
    bf = ml_dtypes.bfloat16

    def chunk(w):
        # stage [128 p, 8 kb, M] with contraction index d = 128*kb + p
        return np.ascontiguousarray(
            w.reshape(KB, P, -1).transpose(1, 0, 2).astype(bf))

    tri = np.triu(np.ones((P, P), np.float32)).astype(bf)

    in_maps = []
    for c in range(N_CORES):
        b, g = c // 4, c % 4
        cols = np.arange(H_LOC * g * DIM_HEAD, H_LOC * (g + 1) * DIM_HEAD)
        wq_s = w_qkv[:, cols]
        wk_s = w_qkv[:, INNER + cols]
        wv_s = w_qkv[:, 2 * INNER + cols]
        wo_s = w_out[cols, :]
        in_maps.append({
            "x": np.ascontiguousarray(x[b]).astype(bf),
            "wq": chunk(ln_w[:, None] * wq_s * SCALE),
            "wk": chunk(ln_w[:, None] * wk_s),
            "wv": chunk(ln_w[:, None] * wv_s),
            "wo": np.ascontiguousarray(
                wo_s.reshape(2, 2, DIM_HEAD, DIM).transpose(1, 2, 0, 3)
                .reshape(P, 2, DIM).astype(bf)),
            "bq": np.ascontiguousarray(((ln_b @ wq_s) * SCALE).reshape(2, P).T),
            "bk": np.ascontiguousarray((ln_b @ wk_s).reshape(2, P).T),
            "bv": (ln_b @ wv_s).reshape(1, H_LOC, DIM_HEAD),
            "tri": tri,
        })
    return in_maps


_NC_CACHE = []


def kernel(x, ln_w, ln_b, w_qkv, w_out):
    in_maps = make_in_maps(x, ln_w, ln_b, w_qkv, w_out)
    if not _NC_CACHE:
        _NC_CACHE.append(build_nc())
    nc = _NC_CACHE[0]
    res = run_bass_kernel_spmd(nc, in_maps, list(range(N_CORES))).results
    out = np.zeros((B, N, DIM), np.float32)
    for c in range(N_CORES):
        out[c // 4] += res[c]["out"]
    return out
